# revision 20
# baseline (speedup 1.0000x reference)
"""BM25 scoring kernel for Trainium2 (8 NeuronCores, SPMD).

score = sum_v term1(qtf_v) * term2(ptf_v) * term3(dfs_v)

term1 is nonzero only at the <=4096 query token ids, so we work
query-position-centric:

  score = sum_i  term2(ptf[t_i]) * term3(dfs[t_i]) / (K3 + qtf[t_i])

where t_i ranges over all 4096 query positions (each unique id t appears
qtf_t times, and term1(q)/q = 1/(K3+q), so the sum telescopes exactly).

Sharding ("route ids to owning shard by token-id range"): the host sorts
the 4096 query ids and cuts the sorted list into 8 cores x 128
partitions of exactly QPAD=4 ids.  Passage ids are routed to the
partition whose value interval contains them (binary search against the
1024 interval lower bounds -- pure range routing).  A duplicated query
value may straddle two adjacent partitions; the kernel fixes qtf/ptf
for such values by also comparing each partition's q slots against its
neighbor partitions' rows (staged by the host as extra columns of the
same table, so one DMA delivers everything).

Device schedule:
  - ONE input DMA loads the combined [128,160] i32 table (qi int32 in
    cols 0:4, f32-bitcast id rows [self|next|prev] in cols 4:160).
  - Four single-column SWDGE indirect gathers of dfs at the qi slots
    (hardware consumes one offset per partition per instruction; the
    serial Q7 descriptor generator is the critical resource).
  - ONE fused is_equal compare over the row triple + two strided
    multi-axis reduces give qtf/ptf; the v weights and per-column BM25
    terms are fused scalar_tensor_tensor ops that interleave under the
    gathers.  The last column needs only Ln + two STT ops after the
    final gather.
  - term3 uses ln(N+0.5-d) = ln(N+0.5) - d/(N+0.5) + O((d/N)^2)
    (d<=1000, N~8.8e6, error <1e-8), so a single Ln per column:
      t3 = Cln - d*invN - ln(d + 0.5)
  - No matmul/PSUM: the kernel writes [128,1] per-partition partials;
    the host sums the 1024 partials and applies the K1/ln2 scale (the
    final sum all-reduce).

Sentinels: pad p slots hold -2, shifted-row padding at the chain ends
holds -3; q slots are all real ids.  A q slot whose value has no
passage match gets ptf=0 so term2 = 0 exactly and its term vanishes.
"""

import math

import numpy as np

import concourse.bacc as bacc
import concourse.bass as bass
import concourse.tile as tile
from concourse import mybir
from concourse.bass_utils import run_bass_kernel_spmd

# ---- problem constants (from the BM25 reference) ----
VOCAB = 8_388_608
NQ = 4096
NP = 8192
K1, K3, B = 1.2, 8.0, 0.75
N_DOCS = 8_841_823.0
L_AVE = 55.0
L_D = NP  # passage length (static)
C2 = K1 * (1.0 - B + B * L_D / L_AVE)  # term2 denominator constant
INV_LN2 = 1.0 / math.log(2.0)
INV_N = 1.0 / (N_DOCS + 0.5)
C_LN = math.log(N_DOCS + 0.5)

NCORES = 8
P = 128
NPART = NCORES * P  # 1024 partitions global
QPAD = 4   # q slots per partition: exactly 4096/1024
PPAD = 48  # p-run slots per partition (seed inputs max ~36)
W = QPAD + PPAD
W3 = 3 * W  # self row + next-neighbor row + prev-neighbor row
NCOL = QPAD + W3  # 160: [qi int32 | self | next | prev]

F32 = mybir.dt.float32
I32 = mybir.dt.int32


def _build_program():
    nc = bacc.Bacc(
        "TRN2", target_bir_lowering=False, debug=False, num_devices=NCORES
    )
    qi = nc.dram_tensor("qi", [P, QPAD], I32, kind="ExternalInput").ap()
    qp = nc.dram_tensor("qp", [P, W3], F32, kind="ExternalInput").ap()
    dfs = nc.dram_tensor("dfs", [VOCAB, 1], F32, kind="ExternalInput").ap()
    partial = nc.dram_tensor("partial", [1, QPAD], F32, kind="ExternalOutput").ap()

    with tile.TileContext(nc) as tc:
        with tc.tile_pool(name="sb", bufs=1) as spool, \
             tc.tile_pool(name="ps", bufs=1, space="PSUM") as ppool:
            # ---- qi first on the scalar HWDGE queue (gates the serial
            # gather chain); qp rides the sync HWDGE queue in parallel.
            qi_t = spool.tile([P, QPAD], I32)
            nc.scalar.dma_start(out=qi_t[:], in_=qi)
            qp_t = spool.tile([P, W3], F32)
            nc.sync.dma_start(out=qp_t[:], in_=qp)

            # tiny SWDGE warm-up: an indirect gather whose offsets are
            # all out-of-bounds (silently skipped) pulls the Q7 indirect
            # descriptor path + ring init out of the gather critical
            # path without moving any data.
            wdm = spool.tile([16, 1], F32)

            half = spool.tile([P, 1], F32)
            nc.vector.memset(half[:], 0.5)
            ones = nc.const_aps.tensor(1.0, (P, 1), F32)
            nc.gpsimd.indirect_dma_start(
                out=wdm[:],
                out_offset=None,
                in_=dfs,
                in_offset=bass.IndirectOffsetOnAxis(
                    ap=half[0:16, 0:1].bitcast(I32), axis=0
                ),
                bounds_check=VOCAB - 1,
                oob_is_err=False,
            )

            # ACT warm-up on the bias tile: forces both Ln table loads
            # to the top of the scalar queue, before any gather dep.
            wm = spool.tile([P, 1], F32)
            nc.scalar.activation(
                wm[:], half[:], mybir.ActivationFunctionType.Ln,
                bias=half[:], scale=1.0,
            )

            # ---- four serial indirect gathers (the critical chain) ----
            dfsg = spool.tile([P, QPAD], F32)
            for k in range(QPAD):
                nc.gpsimd.indirect_dma_start(
                    out=dfsg[:, k : k + 1],
                    out_offset=None,
                    in_=dfs,
                    in_offset=bass.IndirectOffsetOnAxis(
                        ap=qi_t[:, k : k + 1], axis=0
                    ),
                )

            # ---- l_k = Ln(d_k + 0.5) as each gather lands ----
            lt = spool.tile([P, QPAD], F32)
            for k in range(QPAD):
                nc.scalar.activation(
                    lt[:, k : k + 1],
                    dfsg[:, k : k + 1],
                    mybir.ActivationFunctionType.Ln,
                    bias=half[:],
                    scale=1.0,
                )

            # ---- counts: one fused compare + two strided reduces ----
            qp_f = qp_t[:]  # [P, W3]
            q_b = qp_f[:, 0:QPAD].unsqueeze(2).broadcast_to((P, QPAD, W3))
            o_b = qp_f.unsqueeze(1).broadcast_to((P, QPAD, W3))
            mt = spool.tile([P, QPAD, W3], F32)
            nc.vector.tensor_tensor(mt[:], q_b, o_b, mybir.AluOpType.is_equal)

            def mt_view(lo, n):
                # [P, QPAD, 3, n] strided view over mt = [P, QPAD, 3, W]
                v = mt[:, :, 0:1].unsqueeze(2).copy()
                v.ap[2] = [W, 3]
                v.ap[3] = [1, n]
                v.offset = v.offset + lo
                return v

            qc = spool.tile([P, QPAD], F32)
            nc.vector.tensor_reduce(
                out=qc[:], in_=mt_view(0, QPAD),
                axis=mybir.AxisListType.XY, op=mybir.AluOpType.add,
            )
            pc = spool.tile([P, QPAD], F32)
            nc.vector.tensor_reduce(
                out=pc[:], in_=mt_view(QPAD, PPAD),
                axis=mybir.AxisListType.XY, op=mybir.AluOpType.add,
            )

            # ---- v = ptf / ((K3 + qtf) * (C2 + ptf)) ; pv = Cln * v ----
            da = spool.tile([P, QPAD], F32)
            nc.vector.tensor_scalar(
                out=da[:], in0=qc[:], scalar1=float(K3), scalar2=None,
                op0=mybir.AluOpType.add,
            )
            db = spool.tile([P, QPAD], F32)
            nc.vector.tensor_scalar(
                out=db[:], in0=pc[:], scalar1=float(C2), scalar2=None,
                op0=mybir.AluOpType.add,
            )
            nc.vector.tensor_tensor(da[:], da[:], db[:], mybir.AluOpType.mult)
            nc.vector.reciprocal(da[:], da[:])
            vv = spool.tile([P, QPAD], F32)
            nc.vector.tensor_tensor(vv[:], pc[:], da[:], mybir.AluOpType.mult)
            # pv = Cln*v ; niv = -invN*v ; nv = -v   (all gather-free)
            pv = spool.tile([P, QPAD], F32)
            nc.vector.tensor_scalar(
                out=pv[:], in0=vv[:], scalar1=float(C_LN), scalar2=None,
                op0=mybir.AluOpType.mult,
            )
            niv = spool.tile([P, QPAD], F32)
            nc.vector.tensor_scalar(
                out=niv[:], in0=vv[:], scalar1=-float(INV_N), scalar2=None,
                op0=mybir.AluOpType.mult,
            )
            nv = spool.tile([P, QPAD], F32)
            nc.vector.tensor_scalar(
                out=nv[:], in0=vv[:], scalar1=-1.0, scalar2=None,
                op0=mybir.AluOpType.mult,
            )

            # ---- per-column (w_k = v_k*t3_k, two STTs):
            #   t_k = d_k*niv_k + pv_k     (needs only the gather)
            #   w_k = l_k*nv_k + t_k       (needs the Ln)
            # The partition sum happens in ONE matmul (ones^T @ ww ->
            # [1,QPAD] PSUM), so no add chain sits after the last column.
            tt = spool.tile([P, QPAD], F32)
            ww = spool.tile([P, QPAD], F32)
            for k in range(QPAD):
                nc.vector.scalar_tensor_tensor(
                    out=tt[:, k : k + 1],
                    in0=dfsg[:, k : k + 1],
                    scalar=niv[:, k : k + 1],
                    in1=pv[:, k : k + 1],
                    op0=mybir.AluOpType.mult,
                    op1=mybir.AluOpType.add,
                )
                nc.vector.scalar_tensor_tensor(
                    out=ww[:, k : k + 1],
                    in0=lt[:, k : k + 1],
                    scalar=nv[:, k : k + 1],
                    in1=tt[:, k : k + 1],
                    op0=mybir.AluOpType.mult,
                    op1=mybir.AluOpType.add,
                )
            acc = ppool.tile([1, QPAD], F32, space="PSUM")
            nc.tensor.matmul(
                acc[:], lhsT=ones, rhs=ww[:], start=True, stop=True
            )
            res = spool.tile([1, QPAD], F32)
            nc.vector.tensor_copy(res[:], acc[:])
            nc.sync.dma_start(out=partial, in_=res[:])

    nc.compile()
    return nc


_NC_CACHE = None


def _get_program():
    global _NC_CACHE
    if _NC_CACHE is None:
        _NC_CACHE = _build_program()
    return _NC_CACHE


def _layout(q, p):
    """Sorted layout, exactly 4 q ids per partition, with neighbor rows.

    Returns qp_all [NCORES, P, W3] f32 and qi_all [NCORES, P, QPAD] i32.
    """
    qs = np.sort(q)
    _, counts = np.unique(qs, return_counts=True)
    if counts.max() > QPAD:
        raise ValueError(f"query value repeated {counts.max()} times > {QPAD}")

    base = np.full((NPART, W), -2.0, dtype=np.float32)
    base[:, 0:QPAD] = qs.astype(np.float32).reshape(NPART, QPAD)
    qi_all = np.ascontiguousarray(
        qs.astype(np.int32).reshape(NCORES, P, QPAD)
    )

    # route p ids by interval lower bounds (pure range routing)
    lows = qs[0::QPAD]  # 1024 interval lower bounds
    pg = np.searchsorted(lows, p, side="right") - 1
    pg = np.clip(pg, 0, NPART - 1)
    order = np.argsort(pg, kind="stable")
    pgs = pg[order]
    pid = p[order]
    pslot = np.arange(len(p)) - np.searchsorted(pgs, pgs, side="left")
    if pslot.size and pslot.max() >= PPAD:
        raise ValueError(
            f"p-run overflow: occupancy {pslot.max() + 1} > PPAD={PPAD}"
        )
    base[pgs, QPAD + pslot] = pid.astype(np.float32)

    # self row + next row + prev row (global partition chain, -3 ends)
    edge = np.full((1, W), -3.0, dtype=np.float32)
    nxt = np.vstack([base[1:], edge])
    prv = np.vstack([edge, base[:-1]])
    qp_all = np.concatenate([base, nxt, prv], axis=1).reshape(
        NCORES, P, W3
    )
    return np.ascontiguousarray(qp_all), qi_all


def make_in_maps(query_ids, passage_ids, dfs):
    q = np.asarray(query_ids).reshape(-1).astype(np.int64)
    p = np.asarray(passage_ids).reshape(-1).astype(np.int64)
    d = np.ascontiguousarray(
        np.asarray(dfs).reshape(VOCAB, 1).astype(np.float32)
    )
    qp_all, qi_all = _layout(q, p)
    return [
        {"qi": qi_all[c], "qp": qp_all[c], "dfs": d}
        for c in range(NCORES)
    ]


def kernel(query_ids, passage_ids, dfs, **run_kwargs):
    nc = _get_program()
    in_maps = make_in_maps(query_ids, passage_ids, dfs)
    res = run_bass_kernel_spmd(
        nc, in_maps, core_ids=list(range(NCORES)), **run_kwargs
    )
    total = np.float32(
        K1 * INV_LN2
        * float(np.sum([float(r["partial"].sum()) for r in res.results]))
    )
    out = np.array([total], dtype=np.float32)
    kernel.last_results = res
    return out
